# revision 2
# baseline (speedup 1.0000x reference)
"""GQA attention kernel for 8 Trainium2 NeuronCores.

Sharding: core c -> (b = c // 4, kv-group gk = c % 4).
Each core computes, for its batch b and its kv head gk (which owns the 4
contiguous q-heads gk*4..gk*4+3):
    q/k/v projections, attention, and a partial out-projection
    out_partial[b] = o_heads @ Wo[:, gk*512:(gk+1)*512].T
Host sums the 4 partials per batch.

All matmuls in bf16 (fp32 PSUM accumulation). Softmax without max
subtraction (scores are bounded ~|4.5| for this problem scale); row sums
are obtained free via a ones-column appended to V, normalization applied
to the 128-wide per-head output before the out projection.
"""

import sys

sys.path.insert(0, "/opt/trn_rl_repo")

import numpy as np
import ml_dtypes

import concourse.bass as bass
import concourse.mybir as mybir
import concourse.tile as tile
from concourse import bacc
from concourse.bass_utils import run_bass_kernel_spmd
from concourse.masks import make_identity

BF16 = mybir.dt.bfloat16
F32 = mybir.dt.float32
bf16 = ml_dtypes.bfloat16

B, N, E = 2, 2048, 2048
H, D, G = 16, 128, 4          # q heads, head dim, group size
HKV = H // G                   # 4 kv heads
JL = G * D                     # 512 local q-head dims per core
ET = E // 128                  # 16 e-tiles
NT = N // 128                  # 16 n/s tiles
CH = N // 512                  # 4 chunks of 512
SCALE = 1.0 / float(np.sqrt(D))

_cached = {}


def _build():
    nc = bacc.Bacc("TRN2", target_bir_lowering=False, debug=False, num_devices=8)

    xT = nc.dram_tensor("xT", [E, N], BF16, kind="ExternalInput")
    wq = nc.dram_tensor("wq", [E, JL], BF16, kind="ExternalInput")
    wk = nc.dram_tensor("wk", [E, D], BF16, kind="ExternalInput")
    wv = nc.dram_tensor("wv", [E, D], BF16, kind="ExternalInput")
    wo = nc.dram_tensor("wo", [JL, E], BF16, kind="ExternalInput")
    out = nc.dram_tensor("out", [N, E], F32, kind="ExternalOutput")

    with tile.TileContext(nc) as tc:
        with (
            tc.tile_pool(name="const", bufs=1) as cpool,
            tc.tile_pool(name="xp", bufs=1) as xpool,
            tc.tile_pool(name="wp", bufs=1) as wpool,
            tc.tile_pool(name="kvp", bufs=1) as kvpool,
            tc.tile_pool(name="qp", bufs=1) as qpool,
            tc.tile_pool(name="pp", bufs=2) as ppool,
            tc.tile_pool(name="op", bufs=6) as opool,
            tc.tile_pool(name="otp", bufs=1) as otpool,
            tc.tile_pool(name="outp", bufs=4) as outpool,
            tc.tile_pool(name="psA", bufs=3, space="PSUM") as psA,
            tc.tile_pool(name="psB", bufs=3, space="PSUM") as psB,
            tc.tile_pool(name="psT", bufs=2, space="PSUM") as psT,
        ):
            ident = cpool.tile([128, 128], BF16, tag="ident")
            make_identity(nc, ident[:])

            x_sb = xpool.tile([128, ET, N], BF16, tag="x")
            wq_sb = wpool.tile([128, ET, JL], BF16, tag="wq")
            wk_sb = wpool.tile([128, ET, D], BF16, tag="wk")
            wv_sb = wpool.tile([128, ET, D], BF16, tag="wv")
            wo_sb = wpool.tile([128, G, E], BF16, tag="wo")
            kT_sb = kvpool.tile([128, N], BF16, tag="kT")
            v_sb = kvpool.tile([128, NT, 130], BF16, tag="v")
            qT_sb = qpool.tile([128, G, N], BF16, tag="qT")
            oT_sb = otpool.tile([128, G, N], BF16, tag="oT")

            # --- input DMAs (k/v weights first: needed soonest) ---
            for et in range(ET):
                nc.sync.dma_start(wk_sb[:, et, :], wk[et * 128:(et + 1) * 128, :])
                nc.sync.dma_start(wv_sb[:, et, :], wv[et * 128:(et + 1) * 128, :])
            for et in range(ET):
                nc.sync.dma_start(x_sb[:, et, :], xT[et * 128:(et + 1) * 128, :])
            for et in range(ET):
                nc.sync.dma_start(wq_sb[:, et, :], wq[et * 128:(et + 1) * 128, :])
            for jt in range(G):
                nc.sync.dma_start(wo_sb[:, jt, :], wo[jt * 128:(jt + 1) * 128, :])

            # ones column for the fused row-sum trick
            nc.vector.memset(v_sb[:, :, 128:129], 1.0)

            # --- phase 1: kT [d, s], v [s, d], qT [j, n] ---
            for sc in range(CH):
                ps = psA.tile([128, 512], F32, tag="psA")
                for et in range(ET):
                    nc.tensor.matmul(
                        ps[:], wk_sb[:, et, :], x_sb[:, et, sc * 512:(sc + 1) * 512],
                        start=(et == 0), stop=(et == ET - 1),
                    )
                nc.vector.tensor_copy(kT_sb[:, sc * 512:(sc + 1) * 512], ps[:])

            for st in range(NT):
                ps = psB.tile([128, 130], F32, tag="psB")
                for et in range(ET):
                    nc.tensor.matmul(
                        ps[:, 0:128], x_sb[:, et, st * 128:(st + 1) * 128],
                        wv_sb[:, et, :],
                        start=(et == 0), stop=(et == ET - 1),
                    )
                nc.vector.tensor_copy(v_sb[:, st, 0:128], ps[:, 0:128])

            for g in range(G):
                for ncg in range(CH):
                    ps = psA.tile([128, 512], F32, tag="psA")
                    for et in range(ET):
                        nc.tensor.matmul(
                            ps[:], wq_sb[:, et, g * 128:(g + 1) * 128],
                            x_sb[:, et, ncg * 512:(ncg + 1) * 512],
                            start=(et == 0), stop=(et == ET - 1),
                        )
                    nc.vector.tensor_copy(qT_sb[:, g, ncg * 512:(ncg + 1) * 512], ps[:])

            # --- phase 2: attention, software-pipelined over (g, chunk) ---
            def emit_scores(g, c):
                p_t = ppool.tile([128, NT, 512], BF16, tag="p")
                for st in range(NT):
                    ps = psA.tile([128, 512], F32, tag="psA")
                    nc.tensor.matmul(
                        ps[:], kT_sb[:, st * 128:(st + 1) * 128],
                        qT_sb[:, g, c * 512:(c + 1) * 512],
                        start=True, stop=True,
                    )
                    nc.scalar.activation(
                        p_t[:, st, :], ps[:],
                        mybir.ActivationFunctionType.Exp, scale=SCALE,
                    )
                return p_t

            def emit_o(g, c, p_t):
                for t in range(4):
                    pso = psB.tile([128, 130], F32, tag="psB")
                    for st in range(NT):
                        nc.tensor.matmul(
                            pso[:, 0:129], p_t[:, st, t * 128:(t + 1) * 128],
                            v_sb[:, st, 0:129],
                            start=(st == 0), stop=(st == NT - 1),
                        )
                    rc = opool.tile([128, 1], F32, tag="recip")
                    nc.vector.reciprocal(rc[:], pso[:, 128:129])
                    o_n = opool.tile([128, 128], BF16, tag="o_n")
                    nc.vector.tensor_scalar_mul(o_n[:], pso[:, 0:128], rc[:])
                    pst = psT.tile([128, 128], BF16, tag="psT")
                    nc.tensor.transpose(pst[:], o_n[:], ident[:])
                    nc.vector.tensor_copy(
                        oT_sb[:, g, c * 512 + t * 128: c * 512 + (t + 1) * 128],
                        pst[:],
                    )

            prev = None
            for g in range(G):
                for c in range(CH):
                    p_t = emit_scores(g, c)
                    if prev is not None:
                        emit_o(prev[0], prev[1], prev[2])
                    prev = (g, c, p_t)
            emit_o(prev[0], prev[1], prev[2])

            # --- phase 3: partial out projection ---
            for nt in range(NT):
                for ec in range(CH):
                    ps = psA.tile([128, 512], F32, tag="psA")
                    for g in range(G):
                        nc.tensor.matmul(
                            ps[:], oT_sb[:, g, nt * 128:(nt + 1) * 128],
                            wo_sb[:, g, ec * 512:(ec + 1) * 512],
                            start=(g == 0), stop=(g == G - 1),
                        )
                    ot = outpool.tile([128, 512], F32, tag="out")
                    nc.vector.tensor_copy(ot[:], ps[:])
                    nc.sync.dma_start(
                        out[nt * 128:(nt + 1) * 128, ec * 512:(ec + 1) * 512], ot[:]
                    )

    nc.compile()
    return nc


def get_nc():
    if "nc" not in _cached:
        _cached["nc"] = _build()
    return _cached["nc"]


def make_in_maps(x, Wq, Wk, Wv, Wo):
    """Per-core host-side sharding. Core c -> (b=c//4, gk=c%4)."""
    in_maps = []
    xT = [np.ascontiguousarray(x[b].T).astype(bf16) for b in range(B)]
    wq_s = [np.ascontiguousarray(Wq[gk * JL:(gk + 1) * JL, :].T).astype(bf16)
            for gk in range(HKV)]
    wk_s = [np.ascontiguousarray(Wk[gk * D:(gk + 1) * D, :].T).astype(bf16)
            for gk in range(HKV)]
    wv_s = [np.ascontiguousarray(Wv[gk * D:(gk + 1) * D, :].T).astype(bf16)
            for gk in range(HKV)]
    wo_s = [np.ascontiguousarray(Wo[:, gk * JL:(gk + 1) * JL].T).astype(bf16)
            for gk in range(HKV)]
    for c in range(8):
        b, gk = c // 4, c % 4
        in_maps.append({
            "xT": xT[b], "wq": wq_s[gk], "wk": wk_s[gk],
            "wv": wv_s[gk], "wo": wo_s[gk],
        })
    return in_maps


def kernel(x, Wq, Wk, Wv, Wo):
    nc = get_nc()
    in_maps = make_in_maps(x, Wq, Wk, Wv, Wo)
    res = run_bass_kernel_spmd(nc, in_maps, core_ids=list(range(8)))
    out = np.empty((B, N, E), np.float32)
    for b in range(B):
        acc = res.results[b * 4]["out"]
        for gk in range(1, HKV):
            acc = acc + res.results[b * 4 + gk]["out"]
        out[b] = acc
    return out


# revision 8
# speedup vs baseline: 9.6374x; 9.6374x over previous
"""GQA attention kernel for 8 Trainium2 NeuronCores.

Sharding: core c -> (b = c // 4, kv-group gk = c % 4).
Each core computes, for its batch b and its kv head gk (which owns the 4
contiguous q-heads gk*4..gk*4+3):
    q/k/v projections, attention, and a partial out-projection
    out_partial[b] = o_heads @ Wo[:, gk*512:(gk+1)*512].T
Host sums the 4 partials per batch.

All matmuls in bf16 (fp32 PSUM accumulation). Softmax without max
subtraction (scores are bounded ~|4.5| at this problem's weight scale);
row sums come free from a ones-column appended to V; normalization is
applied to the 128-wide per-head output ahead of the out projection.

Layout (per core), everything E/K-major for the PE:
  xT  [E, N]   = x[b].T          kT [128d, N]    scoresT [s, n] chunks
  wq  [E, 512] = Wq rows.T       qT [128, 4g, N]
  wk  [E, 128] = Wk rows.T       v  [128, 16st, 130] (col 128 = ones)
  wv  [E, 128]                   oT [128, 4g, N]
  wo  [512, E] = Wo cols.T       out [N, E] f32 partial
"""

import sys

sys.path.insert(0, "/opt/trn_rl_repo")

import numpy as np
import ml_dtypes

import concourse.bass as bass
import concourse.mybir as mybir
import concourse.tile as tile
from concourse import bacc
from concourse.bass_utils import run_bass_kernel_spmd
from concourse.masks import make_identity

BF16 = mybir.dt.bfloat16
F32 = mybir.dt.float32
bf16 = ml_dtypes.bfloat16

B, N, E = 2, 2048, 2048
H, D, G = 16, 128, 4
HKV = H // G
JL = G * D                     # 512 local q-head dims per core
ET = E // 128                  # 16
NT = N // 128                  # 16
CH = N // 512                  # 4
SCALE = 1.0 / float(np.sqrt(D))

_cached = {}


def _build(iters=1):
    nc = bacc.Bacc("TRN2", target_bir_lowering=False, debug=False, num_devices=8)

    xT = nc.dram_tensor("xT", [E, N], BF16, kind="ExternalInput")
    wq = nc.dram_tensor("wq", [E, JL], BF16, kind="ExternalInput")
    wk = nc.dram_tensor("wk", [E, D], BF16, kind="ExternalInput")
    wv = nc.dram_tensor("wv", [E, D], BF16, kind="ExternalInput")
    wo = nc.dram_tensor("wo", [JL, E], BF16, kind="ExternalInput")
    out = nc.dram_tensor("out", [N, E], F32, kind="ExternalOutput")

    with tile.TileContext(nc) as tc:
        with (
            tc.tile_pool(name="const", bufs=1) as cpool,
            tc.tile_pool(name="xp", bufs=1) as xpool,
            tc.tile_pool(name="wp", bufs=1) as wpool,
            tc.tile_pool(name="kvp", bufs=1) as kvpool,
            tc.tile_pool(name="qp", bufs=1) as qpool,
            tc.tile_pool(name="pp", bufs=2) as ppool,
            tc.tile_pool(name="op", bufs=4) as opool,
            tc.tile_pool(name="otp", bufs=1) as otpool,
            tc.tile_pool(name="outp", bufs=3) as outpool,
            tc.tile_pool(name="ps1", bufs=2, space="PSUM") as P1,
            tc.tile_pool(name="ps2", bufs=4, space="PSUM") as P2,
        ):
            ident = cpool.tile([128, 128], BF16, tag="ident")
            make_identity(nc, ident[:])

            for _ in range(iters):
                _emit_iter(nc, tc, ident, xpool, wpool, kvpool, qpool, ppool,
                           opool, otpool, outpool, P1, P2,
                           xT, wq, wk, wv, wo, out)

    nc.compile()
    return nc


def _emit_iter(nc, tc, ident, xpool, wpool, kvpool, qpool, ppool, opool,
               otpool, outpool, P1, P2, xT, wq, wk, wv, wo, out):
    x_sb = xpool.tile([128, ET, N], BF16, tag="x")
    wq_sb = wpool.tile([128, ET, JL], BF16, tag="wq")
    wk_sb = wpool.tile([128, ET, D], BF16, tag="wk")
    wv_sb = wpool.tile([128, ET, D], BF16, tag="wv")
    wo_sb = wpool.tile([128, G, E], BF16, tag="wo")
    kT_sb = kvpool.tile([128, N], BF16, tag="kT")
    v_sb = kvpool.tile([128, NT, 130], BF16, tag="v")
    qT_sb = qpool.tile([128, G, N], BF16, tag="qT")
    oT_sb = otpool.tile([128, G, N], BF16, tag="oT")

    # --- input DMAs, in consumption order ---
    # wk / wv: single batched DMA each ([E,D] -> [128, ET, D])
    nc.sync.dma_start(wk_sb[:], wk.rearrange("(a p) d -> p a d", p=128))
    nc.sync.dma_start(wv_sb[:], wv.rearrange("(a p) d -> p a d", p=128))
    # x: 8 DMAs of 1MB (two e-tiles each)
    xr = xT.rearrange("(a p) n -> p a n", p=128)
    for i in range(8):
        nc.sync.dma_start(x_sb[:, 2 * i:2 * i + 2, :], xr[:, 2 * i:2 * i + 2, :])
    # wq: 2 DMAs, wo: 4 DMAs
    wqr = wq.rearrange("(a p) j -> p a j", p=128)
    for i in range(2):
        nc.sync.dma_start(wq_sb[:, 8 * i:8 * i + 8, :], wqr[:, 8 * i:8 * i + 8, :])
    for jt in range(G):
        nc.sync.dma_start(wo_sb[:, jt, :], wo[jt * 128:(jt + 1) * 128, :])

    nc.vector.memset(v_sb[:, :, 128:129], 1.0)

    # --- phase 1 ---
    # kT: 4 chunk accumulators (2 double-bank P1 tiles) so the PE can trail
    # the x DMAs; v head tiles on P2 meanwhile.
    kp = [P1.tile([128, 1024], F32, tag="mm1024", name=f"kp{_i}") for _i in range(2)]
    kps = [kp[_i // 2][:, (_i % 2) * 512:(_i % 2 + 1) * 512] for _i in range(CH)]
    vps = [P2.tile([128, 130], F32, tag="oc", name=f"vps{_i}") for _i in range(4)]
    for et in range(ET):
        for sc in range(CH):
            nc.tensor.matmul(
                kps[sc], wk_sb[:, et, :], x_sb[:, et, sc * 512:(sc + 1) * 512],
                start=(et == 0), stop=(et == ET - 1),
            )
        for st in range(4):
            nc.tensor.matmul(
                vps[st][:, 0:128], x_sb[:, et, st * 128:(st + 1) * 128],
                wv_sb[:, et, :],
                start=(et == 0), stop=(et == ET - 1),
            )
    for sc in range(CH):
        nc.vector.tensor_copy(kT_sb[:, sc * 512:(sc + 1) * 512], kps[sc])
    for st in range(4):
        nc.vector.tensor_copy(v_sb[:, st, 0:128], vps[st][:, 0:128])

    # remaining v tiles interleaved with paired q groups
    def emit_v(st):
        ps = P2.tile([128, 130], F32, tag="oc")
        for et in range(ET):
            nc.tensor.matmul(
                ps[:, 0:128], x_sb[:, et, st * 128:(st + 1) * 128],
                wv_sb[:, et, :],
                start=(et == 0), stop=(et == ET - 1),
            )
        nc.vector.tensor_copy(v_sb[:, st, 0:128], ps[:, 0:128])

    def emit_q_pair(q0, q1):
        ps = P1.tile([128, 1024], F32, tag="mm1024")
        for half, (g, ncg) in enumerate((q0, q1)):
            sl = ps[:, half * 512:(half + 1) * 512]
            for et in range(ET):
                nc.tensor.matmul(
                    sl, wq_sb[:, et, g * 128:(g + 1) * 128],
                    x_sb[:, et, ncg * 512:(ncg + 1) * 512],
                    start=(et == 0), stop=(et == ET - 1),
                )
            nc.vector.tensor_copy(qT_sb[:, g, ncg * 512:(ncg + 1) * 512], sl)

    qlist = [(g, ncg) for g in range(G) for ncg in range(CH)]
    vq = list(range(4, NT))
    for i in range(8):
        emit_q_pair(qlist[2 * i], qlist[2 * i + 1])
        for _ in range(2):
            if vq:
                emit_v(vq.pop(0))

    # --- phase 2 + 3, pipelined per chunk of 512 n-columns ---
    # Scores for two s-tiles share one double-bank psum tile so a single
    # (wider, cheaper per element) Exp covers both. o-groups of the
    # previous chunk are interleaved between score pairs to keep the PE
    # busy while ACT digests the exps.
    def emit_score_pair(g, c, p_t, sp):
        ps = P1.tile([128, 1024], F32, tag="mm1024")
        for half in range(2):
            st = 2 * sp + half
            nc.tensor.matmul(
                ps[:, half * 512:(half + 1) * 512],
                kT_sb[:, st * 128:(st + 1) * 128],
                qT_sb[:, g, c * 512:(c + 1) * 512],
                start=True, stop=True,
            )
        nc.scalar.activation(
            p_t[:, 2 * sp * 512:(2 * sp + 2) * 512], ps[:],
            mybir.ActivationFunctionType.Exp, scale=SCALE,
        )

    def emit_o_group(g, c, p_t, t):
        pso = P2.tile([128, 130], F32, tag="oc")
        for st in range(NT):
            nc.tensor.matmul(
                pso[:, 0:129], p_t[:, st * 512 + t * 128: st * 512 + (t + 1) * 128],
                v_sb[:, st, 0:129],
                start=(st == 0), stop=(st == NT - 1),
            )
        rc = opool.tile([128, 1], F32, tag="recip")
        nc.vector.reciprocal(rc[:], pso[:, 128:129])
        o_n = opool.tile([128, 128], BF16, tag="o_n")
        nc.vector.tensor_scalar_mul(o_n[:], pso[:, 0:128], rc[:])
        pst = P2.tile([128, 128], BF16, tag="oc")
        nc.tensor.transpose(pst[:], o_n[:], ident[:])
        nc.vector.tensor_copy(
            oT_sb[:, g, c * 512 + t * 128: c * 512 + (t + 1) * 128], pst[:],
        )

    def emit_out(c):
        for nt in range(4 * c, 4 * c + 4):
            for half in range(2):
                stage = outpool.tile([128, 1024], F32, tag="out")
                ps = P1.tile([128, 1024], F32, tag="mm1024")
                for e2 in range(2):
                    ec = half * 2 + e2
                    for g in range(G):
                        nc.tensor.matmul(
                            ps[:, e2 * 512:(e2 + 1) * 512],
                            oT_sb[:, g, nt * 128:(nt + 1) * 128],
                            wo_sb[:, g, ec * 512:(ec + 1) * 512],
                            start=(g == 0), stop=(g == G - 1),
                        )
                nc.vector.tensor_copy(stage[:], ps[:])
                nc.sync.dma_start(
                    out[nt * 128:(nt + 1) * 128, half * 1024:(half + 1) * 1024],
                    stage[:],
                )

    chunks = [(c, g) for c in range(CH) for g in range(G)]
    prev = None
    for i, (c, g) in enumerate(chunks):
        p_t = ppool.tile([128, NT * 512], BF16, tag="p", name=f"p{i}")
        for sub in range(4):
            emit_score_pair(g, c, p_t, 2 * sub)
            emit_score_pair(g, c, p_t, 2 * sub + 1)
            if prev is not None:
                emit_o_group(prev[0], prev[1], prev[2], sub)
        if i > 0 and i % 4 == 0:
            emit_out(i // 4 - 1)
        prev = (g, c, p_t)
    for sub in range(4):
        emit_o_group(prev[0], prev[1], prev[2], sub)
    emit_out(CH - 1)


def get_nc(iters=1):
    key = ("nc", iters)
    if key not in _cached:
        _cached[key] = _build(iters)
    return _cached[key]


def make_in_maps(x, Wq, Wk, Wv, Wo):
    """Per-core host-side sharding. Core c -> (b=c//4, gk=c%4)."""
    in_maps = []
    xT = [np.ascontiguousarray(x[b].T).astype(bf16) for b in range(B)]
    wq_s = [np.ascontiguousarray(Wq[gk * JL:(gk + 1) * JL, :].T).astype(bf16)
            for gk in range(HKV)]
    wk_s = [np.ascontiguousarray(Wk[gk * D:(gk + 1) * D, :].T).astype(bf16)
            for gk in range(HKV)]
    wv_s = [np.ascontiguousarray(Wv[gk * D:(gk + 1) * D, :].T).astype(bf16)
            for gk in range(HKV)]
    wo_s = [np.ascontiguousarray(Wo[:, gk * JL:(gk + 1) * JL].T).astype(bf16)
            for gk in range(HKV)]
    for c in range(8):
        b, gk = c // 4, c % 4
        in_maps.append({
            "xT": xT[b], "wq": wq_s[gk], "wk": wk_s[gk],
            "wv": wv_s[gk], "wo": wo_s[gk],
        })
    return in_maps


def kernel(x, Wq, Wk, Wv, Wo):
    nc = get_nc()
    in_maps = make_in_maps(x, Wq, Wk, Wv, Wo)
    res = run_bass_kernel_spmd(nc, in_maps, core_ids=list(range(8)))
    out = np.empty((B, N, E), np.float32)
    for b in range(B):
        acc = res.results[b * 4]["out"]
        for gk in range(1, HKV):
            acc = acc + res.results[b * 4 + gk]["out"]
        out[b] = acc
    return out


# revision 9
# speedup vs baseline: 10.5893x; 1.0988x over previous
"""GQA attention kernel for 8 Trainium2 NeuronCores.

Sharding: core c -> (b = c // 4, kv-group gk = c % 4).
Each core computes, for its batch b and its kv head gk (which owns the 4
contiguous q-heads gk*4..gk*4+3):
    q/k/v projections, attention, and a partial out-projection
    out_partial[b] = o_heads @ Wo[:, gk*512:(gk+1)*512].T
Host sums the 4 partials per batch.

All matmuls in bf16 (fp32 PSUM accumulation). Softmax without max
subtraction (scores are bounded ~|4.5| at this problem's weight scale);
row sums come free from a ones-column appended to V; normalization is
applied to the 128-wide per-head output ahead of the out projection.

Layout (per core), everything E/K-major for the PE:
  xT  [E, N]   = x[b].T          kT [128d, N]    scoresT [s, n] chunks
  wq  [E, 512] = Wq rows.T       qT [128, 4g, N]
  wk  [E, 128] = Wk rows.T       v  [128, 16st, 130] (col 128 = ones)
  wv  [E, 128]                   oT [128, 4g, N]
  wo  [512, E] = Wo cols.T       out [N, E] f32 partial
"""

import sys

sys.path.insert(0, "/opt/trn_rl_repo")

import numpy as np
import ml_dtypes

import concourse.bass as bass
import concourse.mybir as mybir
import concourse.tile as tile
from concourse import bacc
from concourse.bass_utils import run_bass_kernel_spmd
from concourse.masks import make_identity

BF16 = mybir.dt.bfloat16
F32 = mybir.dt.float32
bf16 = ml_dtypes.bfloat16

B, N, E = 2, 2048, 2048
H, D, G = 16, 128, 4
HKV = H // G
JL = G * D                     # 512 local q-head dims per core
ET = E // 128                  # 16
NT = N // 128                  # 16
CH = N // 512                  # 4
SCALE = 1.0 / float(np.sqrt(D))

_cached = {}


def _build(iters=1):
    nc = bacc.Bacc("TRN2", target_bir_lowering=False, debug=False, num_devices=8)

    xT = nc.dram_tensor("xT", [E, N], BF16, kind="ExternalInput")
    wq = nc.dram_tensor("wq", [E, JL], BF16, kind="ExternalInput")
    wk = nc.dram_tensor("wk", [E, D], BF16, kind="ExternalInput")
    wv = nc.dram_tensor("wv", [E, D], BF16, kind="ExternalInput")
    wo = nc.dram_tensor("wo", [JL, E], BF16, kind="ExternalInput")
    out = nc.dram_tensor("out", [N, E], F32, kind="ExternalOutput")

    with tile.TileContext(nc) as tc:
        with (
            tc.tile_pool(name="const", bufs=1) as cpool,
            tc.tile_pool(name="xp", bufs=1) as xpool,
            tc.tile_pool(name="wp", bufs=1) as wpool,
            tc.tile_pool(name="kvp", bufs=1) as kvpool,
            tc.tile_pool(name="qp", bufs=1) as qpool,
            tc.tile_pool(name="pp", bufs=2) as ppool,
            tc.tile_pool(name="op", bufs=4) as opool,
            tc.tile_pool(name="otp", bufs=1) as otpool,
            tc.tile_pool(name="outp", bufs=3) as outpool,
            tc.tile_pool(name="ps1", bufs=2, space="PSUM") as P1,
            tc.tile_pool(name="ps2", bufs=4, space="PSUM") as P2,
        ):
            ident = cpool.tile([128, 128], BF16, tag="ident")
            make_identity(nc, ident[:])

            for _ in range(iters):
                _emit_iter(nc, tc, ident, xpool, wpool, kvpool, qpool, ppool,
                           opool, otpool, outpool, P1, P2,
                           xT, wq, wk, wv, wo, out)

    nc.compile()
    return nc


def _emit_iter(nc, tc, ident, xpool, wpool, kvpool, qpool, ppool, opool,
               otpool, outpool, P1, P2, xT, wq, wk, wv, wo, out):
    x_sb = xpool.tile([128, ET, N], BF16, tag="x")
    wq_sb = wpool.tile([128, ET, JL], BF16, tag="wq")
    wk_sb = wpool.tile([128, ET, D], BF16, tag="wk")
    wv_sb = wpool.tile([128, ET, D], BF16, tag="wv")
    wo_sb = wpool.tile([128, G, E], BF16, tag="wo")
    kT_sb = kvpool.tile([128, N], BF16, tag="kT")
    v_sb = kvpool.tile([128, NT, 130], BF16, tag="v")
    qT_sb = qpool.tile([128, G, N], BF16, tag="qT")
    oT_sb = otpool.tile([128, G, N], BF16, tag="oT")

    # --- input DMAs, in consumption order ---
    # wk / wv: single batched DMA each ([E,D] -> [128, ET, D])
    nc.sync.dma_start(wk_sb[:], wk.rearrange("(a p) d -> p a d", p=128))
    nc.scalar.dma_start(wv_sb[:], wv.rearrange("(a p) d -> p a d", p=128))
    # x: 8 DMAs of 1MB (two e-tiles each)
    xr = xT.rearrange("(a p) n -> p a n", p=128)
    for i in range(8):
        eng = nc.sync if i % 2 == 0 else nc.scalar
        eng.dma_start(x_sb[:, 2 * i:2 * i + 2, :], xr[:, 2 * i:2 * i + 2, :])
    # wq: 2 DMAs, wo: 4 DMAs
    wqr = wq.rearrange("(a p) j -> p a j", p=128)
    for i in range(2):
        nc.gpsimd.dma_start(wq_sb[:, 8 * i:8 * i + 8, :], wqr[:, 8 * i:8 * i + 8, :])
    for jt in range(G):
        nc.gpsimd.dma_start(wo_sb[:, jt, :], wo[jt * 128:(jt + 1) * 128, :])

    nc.vector.memset(v_sb[:, :, 128:129], 1.0)

    # --- phase 1 ---
    # kT: 4 chunk accumulators (2 double-bank P1 tiles) so the PE can trail
    # the x DMAs; v head tiles on P2 meanwhile.
    kp = [P1.tile([128, 1024], F32, tag="mm1024", name=f"kp{_i}") for _i in range(2)]
    kps = [kp[_i // 2][:, (_i % 2) * 512:(_i % 2 + 1) * 512] for _i in range(CH)]
    vps = [P2.tile([128, 130], F32, tag="oc", name=f"vps{_i}") for _i in range(4)]
    for et in range(ET):
        for sc in range(CH):
            nc.tensor.matmul(
                kps[sc], wk_sb[:, et, :], x_sb[:, et, sc * 512:(sc + 1) * 512],
                start=(et == 0), stop=(et == ET - 1),
            )
        for st in range(4):
            nc.tensor.matmul(
                vps[st][:, 0:128], x_sb[:, et, st * 128:(st + 1) * 128],
                wv_sb[:, et, :],
                start=(et == 0), stop=(et == ET - 1),
            )
    for sc in range(CH):
        nc.vector.tensor_copy(kT_sb[:, sc * 512:(sc + 1) * 512], kps[sc])
    for st in range(4):
        nc.vector.tensor_copy(v_sb[:, st, 0:128], vps[st][:, 0:128])

    # remaining v tiles interleaved with paired q groups
    def emit_v(st):
        ps = P2.tile([128, 130], F32, tag="oc")
        for et in range(ET):
            nc.tensor.matmul(
                ps[:, 0:128], x_sb[:, et, st * 128:(st + 1) * 128],
                wv_sb[:, et, :],
                start=(et == 0), stop=(et == ET - 1),
            )
        nc.vector.tensor_copy(v_sb[:, st, 0:128], ps[:, 0:128])

    def emit_q_pair(q0, q1):
        ps = P1.tile([128, 1024], F32, tag="mm1024")
        for half, (g, ncg) in enumerate((q0, q1)):
            sl = ps[:, half * 512:(half + 1) * 512]
            for et in range(ET):
                nc.tensor.matmul(
                    sl, wq_sb[:, et, g * 128:(g + 1) * 128],
                    x_sb[:, et, ncg * 512:(ncg + 1) * 512],
                    start=(et == 0), stop=(et == ET - 1),
                )
            nc.vector.tensor_copy(qT_sb[:, g, ncg * 512:(ncg + 1) * 512], sl)

    qlist = [(g, ncg) for g in range(G) for ncg in range(CH)]
    vq = list(range(4, NT))
    for i in range(8):
        emit_q_pair(qlist[2 * i], qlist[2 * i + 1])
        for _ in range(2):
            if vq:
                emit_v(vq.pop(0))

    # --- phase 2 + 3, pipelined per chunk of 512 n-columns ---
    # Scores for two s-tiles share one double-bank psum tile so a single
    # (wider, cheaper per element) Exp covers both. o-groups of the
    # previous chunk are interleaved between score pairs to keep the PE
    # busy while ACT digests the exps.
    def emit_score_pair(g, c, p_t, sp):
        ps = P1.tile([128, 1024], F32, tag="mm1024")
        for half in range(2):
            st = 2 * sp + half
            nc.tensor.matmul(
                ps[:, half * 512:(half + 1) * 512],
                kT_sb[:, st * 128:(st + 1) * 128],
                qT_sb[:, g, c * 512:(c + 1) * 512],
                start=True, stop=True,
            )
        nc.scalar.activation(
            p_t[:, 2 * sp * 512:(2 * sp + 2) * 512], ps[:],
            mybir.ActivationFunctionType.Exp, scale=SCALE,
        )

    def emit_o_group(g, c, p_t, t):
        pso = P2.tile([128, 130], F32, tag="oc")
        for st in range(NT):
            nc.tensor.matmul(
                pso[:, 0:129], p_t[:, st * 512 + t * 128: st * 512 + (t + 1) * 128],
                v_sb[:, st, 0:129],
                start=(st == 0), stop=(st == NT - 1),
            )
        rc = opool.tile([128, 1], F32, tag="recip")
        nc.vector.reciprocal(rc[:], pso[:, 128:129])
        o_n = opool.tile([128, 128], BF16, tag="o_n")
        nc.vector.tensor_scalar_mul(o_n[:], pso[:, 0:128], rc[:])
        pst = P2.tile([128, 128], BF16, tag="oc")
        nc.tensor.transpose(pst[:], o_n[:], ident[:])
        nc.vector.tensor_copy(
            oT_sb[:, g, c * 512 + t * 128: c * 512 + (t + 1) * 128], pst[:],
        )

    def emit_out(c):
        for nt in range(4 * c, 4 * c + 4):
            for half in range(2):
                stage = outpool.tile([128, 1024], F32, tag="out")
                ps = P1.tile([128, 1024], F32, tag="mm1024")
                for e2 in range(2):
                    ec = half * 2 + e2
                    for g in range(G):
                        nc.tensor.matmul(
                            ps[:, e2 * 512:(e2 + 1) * 512],
                            oT_sb[:, g, nt * 128:(nt + 1) * 128],
                            wo_sb[:, g, ec * 512:(ec + 1) * 512],
                            start=(g == 0), stop=(g == G - 1),
                        )
                nc.vector.tensor_copy(stage[:], ps[:])
                eng = nc.sync if (nt + half) % 2 == 0 else nc.scalar
                eng.dma_start(
                    out[nt * 128:(nt + 1) * 128, half * 1024:(half + 1) * 1024],
                    stage[:],
                )

    chunks = [(c, g) for c in range(CH) for g in range(G)]
    prev = None
    for i, (c, g) in enumerate(chunks):
        p_t = ppool.tile([128, NT * 512], BF16, tag="p", name=f"p{i}")
        for sub in range(4):
            emit_score_pair(g, c, p_t, 2 * sub)
            emit_score_pair(g, c, p_t, 2 * sub + 1)
            if prev is not None:
                emit_o_group(prev[0], prev[1], prev[2], sub)
        if i > 0 and i % 4 == 0:
            emit_out(i // 4 - 1)
        prev = (g, c, p_t)
    for sub in range(4):
        emit_o_group(prev[0], prev[1], prev[2], sub)
    emit_out(CH - 1)


def get_nc(iters=1):
    key = ("nc", iters)
    if key not in _cached:
        _cached[key] = _build(iters)
    return _cached[key]


def make_in_maps(x, Wq, Wk, Wv, Wo):
    """Per-core host-side sharding. Core c -> (b=c//4, gk=c%4)."""
    in_maps = []
    xT = [np.ascontiguousarray(x[b].T).astype(bf16) for b in range(B)]
    wq_s = [np.ascontiguousarray(Wq[gk * JL:(gk + 1) * JL, :].T).astype(bf16)
            for gk in range(HKV)]
    wk_s = [np.ascontiguousarray(Wk[gk * D:(gk + 1) * D, :].T).astype(bf16)
            for gk in range(HKV)]
    wv_s = [np.ascontiguousarray(Wv[gk * D:(gk + 1) * D, :].T).astype(bf16)
            for gk in range(HKV)]
    wo_s = [np.ascontiguousarray(Wo[:, gk * JL:(gk + 1) * JL].T).astype(bf16)
            for gk in range(HKV)]
    for c in range(8):
        b, gk = c // 4, c % 4
        in_maps.append({
            "xT": xT[b], "wq": wq_s[gk], "wk": wk_s[gk],
            "wv": wv_s[gk], "wo": wo_s[gk],
        })
    return in_maps


def kernel(x, Wq, Wk, Wv, Wo):
    nc = get_nc()
    in_maps = make_in_maps(x, Wq, Wk, Wv, Wo)
    res = run_bass_kernel_spmd(nc, in_maps, core_ids=list(range(8)))
    out = np.empty((B, N, E), np.float32)
    for b in range(B):
        acc = res.results[b * 4]["out"]
        for gk in range(1, HKV):
            acc = acc + res.results[b * 4 + gk]["out"]
        out[b] = acc
    return out
